# revision 6
# baseline (speedup 1.0000x reference)
"""Trainium2 Bass kernel for BaseLSTMModel (conv -> MLP -> reset-LSTM -> MLP).

Strategy:
  * conv(3x3 SAME) and the early MLP are both linear with no nonlinearity in
    between, so they compose into one effective weight W_eff [3072, 128]
    (computed on host from conv_w/early_w; pure linear algebra).
  * Data-parallel over N=8 envs across 8 NeuronCores (weights replicated).
  * The reset-masked LSTM is restructured on host: `done` partitions each
    env's time axis into independent segments (each starts from c=h=0).
    Sorting segments by length (desc) and permuting the time axis turns the
    recurrence into a ragged batch: step tau processes the first S_tau
    segments, reading h/c from the previous step's output block.  Sequential
    depth drops from T=128 to max segment length, with zero padding within a
    core (columns are a permutation of 0..T-1) and no masks on device.
  * Cores share one SPMD program built for the union schedule (per-step max
    segment count across cores); narrower cores pad with clamped columns.
"""

import sys

for _p in ("/opt/trn_rl_repo",):
    if _p not in sys.path:
        sys.path.insert(0, _p)

from contextlib import ExitStack

import numpy as np

N, T, H, W, C = 8, 128, 32, 32, 3
HID = 128
CONV_F = 32
IN_FEAT = H * W * C  # 3072
P = 128
KC = IN_FEAT // P  # 24
NCORES = 8
GATE_ORDER = [0, 1, 3, 2]  # reference split order i,f,g,o -> device order i,f,o,g

LAST_EXEC_NS = None
LAST_TIMESIM_NS = None
LAST_RESULT = None


# ----------------------------------------------------------------- host math
def _build_weff(conv_w, conv_b, early_w, early_b):
    E = early_w.astype(np.float64).reshape(H, W, CONV_F, HID)
    w = conv_w.astype(np.float64)
    Weff = np.zeros((H, W, C, HID), np.float64)
    for dy in range(3):
        for dx in range(3):
            ylo, yhi = max(0, dy - 1), min(H, H + dy - 1)
            xlo, xhi = max(0, dx - 1), min(W, W + dx - 1)
            Weff[ylo:yhi, xlo:xhi] += np.einsum(
                "cf,pqfh->pqch",
                w[dy, dx],
                E[ylo - dy + 1 : yhi - dy + 1, xlo - dx + 1 : xhi - dx + 1],
            )
    beff = np.einsum("f,pqfh->h", conv_b.astype(np.float64), E) + early_b.astype(
        np.float64
    )
    return Weff.reshape(IN_FEAT, HID).astype(np.float32), beff.astype(np.float32)


def _segments(done_row):
    starts = sorted({0} | {t for t in range(1, T) if done_row[t] != 0})
    segs = []
    for i, s in enumerate(starts):
        e = starts[i + 1] if i + 1 < len(starts) else T
        segs.append((s, e - s))
    segs.sort(key=lambda x: -x[1])
    return segs


def _union_schedule(all_segs):
    Lmax = max(segs[0][1] for segs in all_segs)
    S_star = [
        max(sum(1 for _, L in segs if L > tau) for segs in all_segs)
        for tau in range(Lmax)
    ]
    O = [0]
    for s in S_star:
        O.append(O[-1] + s)
    return Lmax, S_star, O, O[-1]


def _core_layout(segs, Lmax, S_star, O, CTOT):
    perm = np.zeros(CTOT, np.int64)  # col -> source time (padding clamps to 0)
    pos_t = np.zeros(T, np.int64)  # time -> col
    for tau in range(Lmax):
        r = 0
        for s, L in segs:
            if L > tau:
                c = O[tau] + r
                perm[c] = s + tau
                pos_t[s + tau] = c
                r += 1
    last_rank, last_len = next(
        (i, L) for i, (s, L) in enumerate(segs) if s + L == T
    )
    cfin_col = O[last_len - 1] + last_rank
    return perm, pos_t, cfin_col


# ---------------------------------------------------------------- device BIR
def _build_nc(Lmax, S_star, O, CTOT):
    import concourse.bacc as bacc
    import concourse.tile as tile
    from concourse import mybir

    F32 = mybir.dt.float32
    AFT = mybir.ActivationFunctionType
    NB = 512  # max fp32 matmul free dim / psum bank

    nc = bacc.Bacc("TRN2", target_bir_lowering=False)
    xg_d = nc.dram_tensor("xg", [KC, P, CTOT], F32, kind="ExternalInput")
    weff_d = nc.dram_tensor("weff", [KC, P, HID], F32, kind="ExternalInput")
    beff_d = nc.dram_tensor("beff", [P, 1], F32, kind="ExternalInput")
    wx_d = nc.dram_tensor("wx", [P, 4 * HID], F32, kind="ExternalInput")
    wh_d = nc.dram_tensor("wh", [P, 4 * HID], F32, kind="ExternalInput")
    lb_d = nc.dram_tensor("lb", [P, 4], F32, kind="ExternalInput")
    wout_d = nc.dram_tensor("wout", [P, HID], F32, kind="ExternalInput")
    bout_d = nc.dram_tensor("bout", [P, 1], F32, kind="ExternalInput")
    yg_d = nc.dram_tensor("yg", [P, CTOT], F32, kind="ExternalOutput")
    hsg_d = nc.dram_tensor("hsg", [P, CTOT], F32, kind="ExternalOutput")
    csg_d = nc.dram_tensor("csg", [P, CTOT], F32, kind="ExternalOutput")

    nblk = [(b, min(NB, CTOT - b)) for b in range(0, CTOT, NB)]

    with tile.TileContext(nc) as tc, ExitStack() as ctx:
        const = ctx.enter_context(tc.tile_pool(name="const", bufs=1))
        stream = ctx.enter_context(tc.tile_pool(name="stream", bufs=4))
        ps = ctx.enter_context(tc.tile_pool(name="ps", bufs=2, space="PSUM"))
        work = ctx.enter_context(tc.tile_pool(name="work", bufs=3))

        wx_t = const.tile([P, 4 * HID], F32)
        nc.sync.dma_start(out=wx_t, in_=wx_d[:, :])
        wh_t = const.tile([P, 4 * HID], F32)
        nc.sync.dma_start(out=wh_t, in_=wh_d[:, :])
        lb_t = const.tile([P, 4], F32)
        nc.sync.dma_start(out=lb_t, in_=lb_d[:, :])
        beff_t = const.tile([P, 1], F32)
        nc.sync.dma_start(out=beff_t, in_=beff_d[:, :])
        wout_t = const.tile([P, HID], F32)
        nc.sync.dma_start(out=wout_t, in_=wout_d[:, :])
        bout_t = const.tile([P, 1], F32)
        nc.sync.dma_start(out=bout_t, in_=bout_d[:, :])

        feat_t = const.tile([P, CTOT], F32)
        zxg_t = const.tile([P, 4, CTOT], F32)
        hsg_t = const.tile([P, CTOT], F32)
        csg_t = const.tile([P, CTOT], F32)
        y_t = const.tile([P, CTOT], F32)

        # phase 1: feat = relu(Weff.T @ xg + beff)   [hid, col]
        assert CTOT <= NB, f"CTOT={CTOT} exceeds single-block limit"
        pf = ps.tile([P, CTOT], F32, tag="pf")
        for k in range(KC):
            wk = stream.tile([P, HID], F32, tag="wk")
            nc.sync.dma_start(out=wk, in_=weff_d[k, :, :])
            xk = stream.tile([P, CTOT], F32, tag="xk")
            nc.sync.dma_start(out=xk, in_=xg_d[k, :, :])
            nc.tensor.matmul(pf, wk, xk, start=(k == 0), stop=(k == KC - 1))
        nc.scalar.activation(feat_t, pf, AFT.Relu, bias=beff_t[:, 0:1])

        # phase 2: zxg[g] = Wx_g.T @ feat + lstm_b_g   [hid_gate, col]
        for g in range(4):
            for b, w in nblk:
                pz = ps.tile([P, NB], F32, tag="pz")
                nc.tensor.matmul(
                    pz[:, :w], wx_t[:, g * HID : (g + 1) * HID], feat_t[:, b : b + w]
                )
                nc.vector.tensor_scalar_add(
                    zxg_t[:, g, b : b + w], pz[:, :w], lb_t[:, g : g + 1]
                )

        # phase 3: segmented recurrence (gate order i,f,o,g)
        for tau in range(Lmax):
            S = S_star[tau]
            o = O[tau]
            if tau == 0:
                z_src = zxg_t[:, :, 0:S]
            else:
                op = O[tau - 1]
                pzr = ps.tile([P, 4, S], F32, tag="pzr")
                for g in range(4):
                    nc.tensor.matmul(
                        pzr[:, g, :],
                        wh_t[:, g * HID : (g + 1) * HID],
                        hsg_t[:, op : op + S],
                    )
                zr = work.tile([P, 4, S], F32, tag="zr")
                nc.vector.tensor_add(zr, pzr, zxg_t[:, :, o : o + S])
                z_src = zr
            gt = work.tile([P, 4, S], F32, tag="gt")
            nc.scalar.activation(gt[:, 0:3, :], z_src[:, 0:3, :], AFT.Sigmoid)
            nc.scalar.activation(gt[:, 3, :], z_src[:, 3, :], AFT.Tanh)
            if tau == 0:
                nc.vector.tensor_mul(csg_t[:, o : o + S], gt[:, 0, :], gt[:, 3, :])
            else:
                t1 = work.tile([P, S], F32, tag="t1")
                nc.vector.tensor_mul(t1, gt[:, 1, :], csg_t[:, op : op + S])
                t2 = work.tile([P, S], F32, tag="t2")
                nc.vector.tensor_mul(t2, gt[:, 0, :], gt[:, 3, :])
                nc.vector.tensor_add(csg_t[:, o : o + S], t1, t2)
            tct = work.tile([P, S], F32, tag="tct")
            nc.scalar.activation(tct, csg_t[:, o : o + S], AFT.Tanh)
            nc.vector.tensor_mul(hsg_t[:, o : o + S], gt[:, 2, :], tct)

        # phase 4: y = relu(Wout.T @ hs + bout)
        for b, w in nblk:
            py = ps.tile([P, NB], F32, tag="py")
            nc.tensor.matmul(py[:, :w], wout_t, hsg_t[:, b : b + w])
            nc.scalar.activation(
                y_t[:, b : b + w], py[:, :w], AFT.Relu, bias=bout_t[:, 0:1]
            )

        nc.sync.dma_start(out=yg_d[:, :], in_=y_t)
        nc.sync.dma_start(out=hsg_d[:, :], in_=hsg_t)
        nc.sync.dma_start(out=csg_d[:, :], in_=csg_t)

    nc.compile()
    return nc


# -------------------------------------------------------------------- driver
def kernel(x, done, conv_w, conv_b, early_w, early_b, lstm_wx, lstm_wh, lstm_b,
           out_w, out_b):
    global LAST_EXEC_NS, LAST_RESULT
    x = np.asarray(x, np.float32)
    done = np.asarray(done)
    Weff, beff = _build_weff(
        np.asarray(conv_w, np.float32), np.asarray(conv_b, np.float32),
        np.asarray(early_w, np.float32), np.asarray(early_b, np.float32))

    all_segs = [_segments(done[n]) for n in range(N)]
    Lmax, S_star, O, CTOT = _union_schedule(all_segs)
    layouts = [_core_layout(all_segs[n], Lmax, S_star, O, CTOT) for n in range(N)]

    gcols = np.concatenate([np.arange(g * HID, (g + 1) * HID) for g in GATE_ORDER])
    wx_r = np.ascontiguousarray(np.asarray(lstm_wx, np.float32)[:, gcols])
    wh_r = np.ascontiguousarray(np.asarray(lstm_wh, np.float32)[:, gcols])
    lb_r = np.ascontiguousarray(
        np.asarray(lstm_b, np.float32)[gcols].reshape(4, HID).T)
    weff_np = np.ascontiguousarray(Weff.reshape(KC, P, HID))
    beff_np = np.ascontiguousarray(beff.reshape(P, 1))
    wout_np = np.ascontiguousarray(np.asarray(out_w, np.float32))
    bout_np = np.ascontiguousarray(np.asarray(out_b, np.float32).reshape(P, 1))

    in_maps = []
    for n in range(N):
        perm, _, _ = layouts[n]
        xt = x[n].reshape(T, IN_FEAT)
        xg = np.ascontiguousarray(xt[perm].T.reshape(KC, P, CTOT))
        in_maps.append({
            "xg": xg, "weff": weff_np, "beff": beff_np, "wx": wx_r, "wh": wh_r,
            "lb": lb_r, "wout": wout_np, "bout": bout_np,
        })

    nc = _build_nc(Lmax, S_star, O, CTOT)

    from concourse.bass_utils import run_bass_kernel_spmd

    import os
    global LAST_TIMESIM_NS
    if os.environ.get("KERNEL_TIMESIM", "0") == "1":
        try:
            from concourse.timeline_sim import TimelineSim
            LAST_TIMESIM_NS = TimelineSim(nc).simulate()
        except Exception as ex:  # noqa: BLE001
            LAST_TIMESIM_NS = None
            print("timesim failed:", ex)
    res = run_bass_kernel_spmd(nc, in_maps, core_ids=list(range(NCORES)))
    LAST_EXEC_NS = res.exec_time_ns
    LAST_RESULT = res

    y = np.empty((N, T, HID), np.float32)
    h_fin = np.empty((N, HID), np.float32)
    c_fin = np.empty((N, HID), np.float32)
    for n in range(N):
        perm, pos_t, cfin_col = layouts[n]
        r = res.results[n]
        y[n] = r["yg"][:, pos_t].T
        h_fin[n] = r["hsg"][:, pos_t[T - 1]]
        c_fin[n] = r["csg"][:, cfin_col]
    return c_fin, h_fin, y
